# revision 7
# baseline (speedup 1.0000x reference)
"""ConvCrossAttention Trainium2 kernel (Bass/Tile), SPMD over 8 NeuronCores.

Sharding: pure data-parallel over batch (B=16 -> 2 images per core). Each core
runs the full two-stream cross-attention block for its 2 images; no collectives.

Per-core algorithm (all layouts chosen so no transposes are ever needed):
  - depthwise 3x3 conv (+folded BN scale) via 9 fused scalar_tensor_tensor
    taps; channels on partitions, spatial flat on free dim. Full-res (q path)
    taps run on DVE in bf16 (2x mode); strided kv taps run on GpSimd (their
    stride-2 access can't use DVE 2x mode anyway).
  - pointwise convs on PE (bf16 operands, fp32 PSUM):
      Q:  [cin,pos] x WqT  -> qT [cout(head-major),pos]
      K:  [cin,pos] x WkvT -> kT [cout,pos]
      V:  x-stationary     -> v  [pos, cout]   (needed for attn@v contraction)
  - attention per head: dotsT = kT.T@qT on PE (kv on partitions, q on free),
    exp on ACT (logits are tiny -> no max subtraction) -> bf16 p, denominator
    via col-tiled ones-matmuls on PE + DVE reciprocal, oT = v.T@p on PE with
    head pairs packed in 128 partitions; normalization fused into the
    PSUM->SBUF eviction (DVE mult) -> bf16 oT.
  - V conv bias is folded into the output conv bias on the host (softmax rows
    sum to exactly 1), BN is folded into depthwise taps + pointwise bias.
  - output 1x1 conv on PE + bias on ACT, DMA out fp32.
"""

import numpy as np
import ml_dtypes
from contextlib import ExitStack

import concourse.bass as bass
import concourse.bacc as bacc
import concourse.tile as tile
import concourse.mybir as mybir
from concourse.bass_utils import run_bass_kernel_spmd

F32 = mybir.dt.float32
BF16 = mybir.dt.bfloat16
NPBF16 = ml_dtypes.bfloat16
AOP = mybir.AluOpType
AF = mybir.ActivationFunctionType

N_CORES = 8
B = 16
IMGS = B // N_CORES          # images per core
DIM = 256                    # conv in channels
HEADS, DH, INNER = 8, 64, 512
HW = 1024                    # 32*32 q positions
HWK = 256                    # 16*16 kv positions
EPS = 1e-5
SCALE = DH ** -0.5


def _r(ap):
    return ap


# ---------------------------------------------------------------------------
# device kernel
# ---------------------------------------------------------------------------

def _build_module():
    nc = bacc.Bacc("TRN2", target_bir_lowering=False, debug=False)

    def inp(name, shape, dt=F32):
        return nc.dram_tensor(name, shape, dt, kind="ExternalInput")

    x_d = [inp(f"xs{s}", [IMGS, DIM, 34 * 34], BF16) for s in range(2)]
    dgq_d = inp("dgq", [2, 128, 2, 9, 128], BF16)    # diag(dw), p-major for contiguous DMA
    dgkv_d = inp("dgkv", [2, 128, 2, 9, 128], BF16)
    wq_d = [inp(f"wq{s}", [DIM, INNER], BF16) for s in range(2)]      # [cin, cout]
    wkv_d = [inp(f"wkv{s}", [DIM, 2 * INNER], BF16) for s in range(2)]
    bq_d = [inp(f"bq{s}", [4, 128]) for s in range(2)]          # cout chunk-major
    bk_d = [inp(f"bk{s}", [4, 128]) for s in range(2)]
    wo_d = [inp(f"wo{s}", [INNER, DIM], BF16) for s in range(2)]      # [hd, c]
    bo_d = [inp(f"bo{s}", [2, 128]) for s in range(2)]
    ones_d = inp("ones_in", [128, 192], BF16)
    out_d = nc.dram_tensor("out", [2, IMGS, DIM, HW], F32, kind="ExternalOutput")

    with tile.TileContext(nc) as tc, ExitStack() as ctx:
        const = ctx.enter_context(tc.tile_pool(name="const", bufs=1))
        xpool = ctx.enter_context(tc.tile_pool(name="xpool", bufs=2))
        yqpool = ctx.enter_context(tc.tile_pool(name="yqpool", bufs=2))
        ykpool = ctx.enter_context(tc.tile_pool(name="ykpool", bufs=2))
        qTpool = ctx.enter_context(tc.tile_pool(name="qTpool", bufs=8))
        kTpool = ctx.enter_context(tc.tile_pool(name="kTpool", bufs=8))
        vpool = ctx.enter_context(tc.tile_pool(name="vpool", bufs=4))
        pTpool = ctx.enter_context(tc.tile_pool(name="pTpool", bufs=16))
        Rpool = ctx.enter_context(tc.tile_pool(name="Rpool", bufs=2))
        oTpool = ctx.enter_context(tc.tile_pool(name="oTpool", bufs=4))
        outpool = ctx.enter_context(tc.tile_pool(name="outpool", bufs=2))
        psA = ctx.enter_context(tc.tile_pool(name="psA", bufs=2, space="PSUM"))
        psD = ctx.enter_context(tc.tile_pool(name="psD", bufs=2, space="PSUM"))
        psB = ctx.enter_context(tc.tile_pool(name="psB", bufs=2, space="PSUM"))

        # ---- constants -----------------------------------------------------
        wq_sb, wkv_sb, wo_sb, dwq_sb, dwkv_sb, bq_sb, bk_sb, bo_sb = (
            [], [], [], [], [], [], [], [])
        for s in range(2):
            wq_sb.append([const.tile([128, INNER], BF16, tag=f"wq{s}{k}", name=f"wq{s}{k}")
                          for k in range(2)])
            wkv_sb.append([const.tile([128, 2 * INNER], BF16, tag=f"wkv{s}{k}", name=f"wkv{s}{k}")
                           for k in range(2)])
            wo_sb.append([const.tile([128, DIM], BF16, tag=f"wo{s}{m}", name=f"wo{s}{m}")
                          for m in range(4)])
            for k in range(2):
                nc.sync.dma_start(out=wq_sb[s][k][:],
                                  in_=wq_d[s].ap()[k * 128:(k + 1) * 128, :])
                nc.sync.dma_start(out=wkv_sb[s][k][:],
                                  in_=wkv_d[s].ap()[k * 128:(k + 1) * 128, :])
            for m in range(4):
                nc.sync.dma_start(out=wo_sb[s][m][:],
                                  in_=wo_d[s].ap()[m * 128:(m + 1) * 128, :])
            dwq_sb.append(const.tile([128, 2, 9, 128], BF16, tag=f"dgq{s}", name=f"dgq{s}"))
            dwkv_sb.append(const.tile([128, 2, 9, 128], BF16, tag=f"dgkv{s}", name=f"dgkv{s}"))
            nc.sync.dma_start(out=dwq_sb[s][:], in_=dgq_d.ap()[s])
            nc.sync.dma_start(out=dwkv_sb[s][:], in_=dgkv_d.ap()[s])
            bq_sb.append(const.tile([128, 4], F32, tag=f"bq{s}", name=f"bq{s}"))
            bk_sb.append(const.tile([128, 4], F32, tag=f"bk{s}", name=f"bk{s}"))
            bo_sb.append(const.tile([128, 2], F32, tag=f"bo{s}", name=f"bo{s}"))
            nc.sync.dma_start(out=bq_sb[s][:], in_=bq_d[s].ap().rearrange("m p -> p m"))
            nc.sync.dma_start(out=bk_sb[s][:], in_=bk_d[s].ap().rearrange("m p -> p m"))
            nc.sync.dma_start(out=bo_sb[s][:], in_=bo_d[s].ap().rearrange("m p -> p m"))
        ones_sb = const.tile([128, 192], BF16, tag="ones", name="ones")
        nc.sync.dma_start(out=ones_sb[:], in_=ones_d.ap())

        for img in range(IMGS):
            qT, kT, v = {}, {}, {}
            # ---- projections for both streams ------------------------------
            for s in range(2):
                # load x, depthwise
                yq, ykv = [], []
                for c in range(2):
                    x_t = xpool.tile([128, 34 * 34], BF16, tag="x", name="x")
                    nc.sync.dma_start(out=x_t[:],
                                      in_=x_d[s].ap()[img, c * 128:(c + 1) * 128, :])
                    x3 = x_t[:].rearrange("p (r q) -> p r q", r=34)
                    # depthwise 3x3 = 9 shifted diag-matmuls accumulating in PSUM
                    yq_t = yqpool.tile([128, HW], BF16, tag="yq", name="yq")
                    for half in range(2):
                        ps = psA.tile([128, 512], F32, tag="mm", name="mm")
                        for t in range(9):
                            ky, kx = t // 3, t % 3
                            nc.tensor.matmul(
                                out=ps[:],
                                lhsT=_r(dwq_sb[s][:, c, t, :]),
                                rhs=_r(x3[:, ky + 16 * half:ky + 16 * half + 16,
                                          kx:kx + 32]),
                                start=(t == 0), stop=(t == 8))
                        nc.vector.tensor_copy(yq_t[:, half * 512:(half + 1) * 512],
                                              ps[:])
                    yq.append(yq_t)

                    ykv_t = ykpool.tile([128, HWK], BF16, tag="ykv", name="ykv")
                    ps = psA.tile([128, 512], F32, tag="mm", name="mm")
                    for t in range(9):
                        ky, kx = t // 3, t % 3
                        nc.tensor.matmul(
                            out=ps[:, 0:256],
                            lhsT=_r(dwkv_sb[s][:, c, t, :]),
                            rhs=_r(x3[:, ky:ky + 32:2, kx:kx + 32:2]),
                            start=(t == 0), stop=(t == 8))
                    nc.vector.tensor_copy(ykv_t[:], ps[:, 0:256])
                    ykv.append(ykv_t)

                # Q pointwise: qT[m] [128, 1024]; bias eviction split ACT/DVE
                for m in range(4):
                    qT_t = qTpool.tile([128, HW], BF16, tag="qT", name="qT")
                    for qh in range(2):
                        ps = psA.tile([128, 512], F32, tag="mm", name="mm")
                        for k in range(2):
                            nc.tensor.matmul(
                                out=ps[:],
                                lhsT=_r(wq_sb[s][k][:, m * 128:(m + 1) * 128]),
                                rhs=_r(yq[k][:, qh * 512:(qh + 1) * 512]),
                                start=(k == 0), stop=(k == 1))
                        if qh == 0:
                            nc.scalar.activation(
                                out=qT_t[:, qh * 512:(qh + 1) * 512], in_=ps[:],
                                func=AF.Identity, bias=bq_sb[s][:, m:m + 1], scale=1.0)
                        else:
                            nc.vector.tensor_scalar_add(
                                qT_t[:, qh * 512:(qh + 1) * 512], ps[:],
                                bq_sb[s][:, m:m + 1])
                    qT[(s, m)] = qT_t
                # K pointwise: kT[m] [128, 256]
                for m in range(4):
                    ps = psA.tile([128, HWK], F32, tag="mm", name="mm")
                    for k in range(2):
                        nc.tensor.matmul(
                            out=ps[:],
                            lhsT=_r(wkv_sb[s][k][:, m * 128:(m + 1) * 128]),
                            rhs=_r(ykv[k][:]),
                            start=(k == 0), stop=(k == 1))
                    kT_t = kTpool.tile([128, HWK], BF16, tag="kT", name="kT")
                    nc.scalar.activation(out=kT_t[:], in_=ps[:], func=AF.Identity,
                                         bias=bk_sb[s][:, m:m + 1], scale=1.0)
                    kT[(s, m)] = kT_t
                # V pointwise (x-stationary): v[p] [128 pos, 512 cout]
                for p in range(2):
                    ps = psA.tile([128, 512], F32, tag="mm", name="mm")
                    for k in range(2):
                        nc.tensor.matmul(
                            out=ps[:],
                            lhsT=_r(ykv[k][:, p * 128:(p + 1) * 128]),
                            rhs=_r(wkv_sb[s][k][:, INNER:2 * INNER]),
                            start=(k == 0), stop=(k == 1))
                    v_t = vpool.tile([128, 512], BF16, tag="v", name="v")
                    nc.vector.tensor_copy(v_t[:], ps[:])
                    vo_t = vpool.tile([128, 512], BF16, tag="vo", name="vo")
                    vo4 = vo_t.rearrange("p (b w) -> p b w", b=4)
                    ps4 = ps.rearrange("p (b w) -> p b w", b=4)
                    nc.vector.tensor_scalar_mul(vo4[:, :, 0:64], ps4[:, :, 0:64], 0.0)
                    nc.vector.tensor_copy(vo4[:, :, 64:128], ps4[:, :, 64:128])
                    v[(s, p)] = v_t
                    v[(s, p, 'odd')] = vo_t

            # ---- attention blocks ------------------------------------------
            for a in range(2):          # output stream a: q from a, k/v from 1-a
                b = 1 - a
                pT = {}
                # dotsT + exp, per head pair
                for hp in range(4):
                    for kc in range(2):
                        dp = [psB.tile([128, HW], F32, tag="big", name="big") for _ in range(2)]
                        for j in range(2):
                            for qh in range(2):
                                nc.tensor.matmul(
                                    out=dp[j][:, qh * 512:(qh + 1) * 512],
                                    lhsT=_r(kT[(b, hp)][64 * j:64 * (j + 1),
                                                        kc * 128:(kc + 1) * 128]),
                                    rhs=_r(qT[(a, hp)][64 * j:64 * (j + 1),
                                                       qh * 512:(qh + 1) * 512]),
                                    start=True, stop=True)
                        for j in range(2):
                            pT_t = pTpool.tile([128, HW], BF16, tag="pT", name="pT")
                            nc.scalar.activation(out=pT_t[:], in_=dp[j][:],
                                                 func=AF.Exp, scale=SCALE)
                            pT[(2 * hp + j, kc)] = pT_t
                # denominators: zero-padded ones matmuls -> D replicated in
                # pair layout (rows 0-63 head 2hp, 64-127 head 2hp+1),
                # reciprocal straight off PSUM into the eviction multiplier
                dr = {}
                for hp in range(4):
                    dr_t = Rpool.tile([128, HW], F32, tag="R", name="R", bufs=3)
                    for qh in range(2):
                        d_t = psD.tile([128, 512], F32, tag="d", name="d")
                        # group: odd kc0 (start, writes all rows incl zeros),
                        # even kc0/kc1 (rows 0-63), odd kc1 (stop, all rows)
                        nc.tensor.matmul(
                            out=d_t[:], lhsT=_r(ones_sb[:, 64:192]),
                            rhs=_r(pT[(2 * hp + 1, 0)][:, qh * 512:(qh + 1) * 512]),
                            start=True, stop=False)
                        for kc in range(2):
                            nc.tensor.matmul(
                                out=d_t[0:64, :], lhsT=_r(ones_sb[:, 0:64]),
                                rhs=_r(pT[(2 * hp, kc)][:, qh * 512:(qh + 1) * 512]),
                                start=False, stop=False)
                        nc.tensor.matmul(
                            out=d_t[:], lhsT=_r(ones_sb[:, 64:192]),
                            rhs=_r(pT[(2 * hp + 1, 1)][:, qh * 512:(qh + 1) * 512]),
                            start=False, stop=True)
                        nc.vector.reciprocal_approx_fast(
                            out=dr_t[:, qh * 512:(qh + 1) * 512], in_=d_t[:])
                    dr[hp] = dr_t
                # oT per pair, odd head (zero-padded M=128, rows 64-127) first,
                # even head (M=64, rows 0-63) accumulates onto its zeros;
                # normalization fused into eviction
                oT = {}
                for hp in range(4):
                    po = psB.tile([128, HW], F32, tag="big", name="big")
                    for qh in range(2):
                        nc.tensor.matmul(
                            out=po[:, qh * 512:(qh + 1) * 512],
                            lhsT=_r(v[(b, 0, 'odd')][:, 128 * hp:128 * (hp + 1)]),
                            rhs=_r(pT[(2 * hp + 1, 0)][:, qh * 512:(qh + 1) * 512]),
                            start=True, stop=False)
                        for kc in range(2):
                            nc.tensor.matmul(
                                out=po[0:64, qh * 512:(qh + 1) * 512],
                                lhsT=_r(v[(b, kc)][:, 128 * hp:128 * hp + 64]),
                                rhs=_r(pT[(2 * hp, kc)][:, qh * 512:(qh + 1) * 512]),
                                start=False, stop=False)
                        nc.tensor.matmul(
                            out=po[:, qh * 512:(qh + 1) * 512],
                            lhsT=_r(v[(b, 1, 'odd')][:, 128 * hp:128 * (hp + 1)]),
                            rhs=_r(pT[(2 * hp + 1, 1)][:, qh * 512:(qh + 1) * 512]),
                            start=False, stop=True)
                    oT_t = oTpool.tile([128, HW], BF16, tag="oT", name="oT")
                    nc.vector.tensor_mul(oT_t[:], po[:], dr[hp][:])
                    oT[hp] = oT_t
                # output 1x1 conv + bias
                for cc in range(2):
                    out_t = outpool.tile([128, HW], F32, tag="out", name="out")
                    for qh in range(2):
                        ps = psA.tile([128, 512], F32, tag="mm", name="mm")
                        for hp in range(4):
                            nc.tensor.matmul(
                                out=ps[:],
                                lhsT=_r(wo_sb[a][hp][:, cc * 128:(cc + 1) * 128]),
                                rhs=_r(oT[hp][:, qh * 512:(qh + 1) * 512]),
                                start=(hp == 0), stop=(hp == 3))
                        nc.scalar.activation(
                            out=out_t[:, qh * 512:(qh + 1) * 512], in_=ps[:],
                            func=AF.Identity, bias=bo_sb[a][:, cc:cc + 1], scale=1.0)
                    nc.sync.dma_start(
                        out=out_d.ap()[a, img, cc * 128:(cc + 1) * 128, :],
                        in_=out_t[:])
    nc.compile()
    return nc


_MODULE = None


def _get_module():
    global _MODULE
    if _MODULE is None:
        _MODULE = _build_module()
    return _MODULE


# ---------------------------------------------------------------------------
# host side: BN folding + sharding + launch
# ---------------------------------------------------------------------------

def _fold(inputs, p):
    dw = np.asarray(inputs[p + '_dw'], np.float32)[:, 0]        # [256,3,3]
    g = np.asarray(inputs[p + '_g'], np.float32)
    b_ = np.asarray(inputs[p + '_b'], np.float32)
    rm = np.asarray(inputs[p + '_rm'], np.float32)
    rv = np.asarray(inputs[p + '_rv'], np.float32)
    pw = np.asarray(inputs[p + '_pw'], np.float32)[:, :, 0, 0]  # [cout, 256]
    inv = g / np.sqrt(rv + EPS)
    dw_eff = (dw * inv[:, None, None]).reshape(DIM, 9)
    bias = pw @ (b_ - rm * inv)
    return dw_eff, pw.T.copy(), bias                             # WT [256, cout]


def host_arrays(inputs):
    """Folded per-core-constant DRAM tensors (same on every core)."""
    h = {'ones_in': np.concatenate([np.ones((128, 64), np.float32),
                                np.zeros((128, 64), np.float32),
                                np.ones((128, 64), np.float32)], axis=1)}
    bf16_keys = {'ones_in', 'dgq', 'dgkv'}
    bv = {}
    dgq, dgkv = {}, {}
    for s, qp, kvp in ((0, 'q1', 'kv1'), (1, 'q2', 'kv2')):
        dwq, WqT, bq = _fold(inputs, qp)
        dwkv, WkvT, bkv = _fold(inputs, kvp)
        dgq[s] = dwq.reshape(2, 128, 9)
        dgkv[s] = dwkv.reshape(2, 128, 9)
        h[f'wq{s}'] = np.ascontiguousarray(WqT)
        h[f'wkv{s}'] = np.ascontiguousarray(WkvT)
        bf16_keys |= {f'wq{s}', f'wkv{s}'}
        h[f'bq{s}'] = bq.reshape(4, 128)
        h[f'bk{s}'] = bkv[:INNER].reshape(4, 128)
        bv[s] = bkv[INNER:]
    for nm, dg in (('dgq', dgq), ('dgkv', dgkv)):
        arr = np.zeros((2, 2, 9, 128, 128), np.float32)
        for s in range(2):
            for c in range(2):
                for t in range(9):
                    np.fill_diagonal(arr[s, c, t], dg[s][c, :, t])
        h[nm] = np.ascontiguousarray(arr.transpose(0, 3, 1, 2, 4))
    for s, op in ((0, 'out1'), (1, 'out2')):
        Wout = np.asarray(inputs[op + '_w'], np.float32)[:, :, 0, 0]  # [256, 512]
        bo = np.asarray(inputs[op + '_b'], np.float32) + Wout @ bv[1 - s]
        h[f'wo{s}'] = np.ascontiguousarray(Wout.T)
        bf16_keys.add(f'wo{s}')
        h[f'bo{s}'] = bo.reshape(2, 128)
    return {k: np.ascontiguousarray(a, dtype=(NPBF16 if k in bf16_keys else np.float32))
            for k, a in h.items()}


def make_in_maps(inputs):
    h = host_arrays(inputs)
    def pad_x(a):
        a = np.asarray(a, np.float32).reshape(B, DIM, 32, 32)
        p = np.zeros((B, DIM, 34, 34), np.float32)
        p[:, :, 1:33, 1:33] = a
        return p.reshape(B, DIM, 34 * 34).astype(NPBF16)
    x1 = pad_x(inputs['x1'])
    x2 = pad_x(inputs['x2'])
    maps = []
    for c in range(N_CORES):
        m = dict(h)
        m['xs0'] = np.ascontiguousarray(x1[c * IMGS:(c + 1) * IMGS])
        m['xs1'] = np.ascontiguousarray(x2[c * IMGS:(c + 1) * IMGS])
        maps.append(m)
    return maps


def gather_out(core_outs):
    """core_outs: list of [2, IMGS, 256, 1024] -> [2, B, 256, 32, 32]."""
    full = np.concatenate([np.asarray(o) for o in core_outs], axis=1)
    return np.ascontiguousarray(full.reshape(2, B, DIM, 32, 32).astype(np.float32))


def kernel(**inputs):
    nc = _get_module()
    in_maps = make_in_maps(inputs)
    res = run_bass_kernel_spmd(nc, in_maps, list(range(N_CORES)))
    return gather_out([r['out'] for r in res.results])


if __name__ == '__main__':
    nc = _build_module()
    print("module built OK")


# revision 9
# speedup vs baseline: 1.1840x; 1.1840x over previous
"""ConvCrossAttention Trainium2 kernel (Bass/Tile), SPMD over 8 NeuronCores.

Sharding: pure data-parallel over batch (B=16 -> 2 images per core). Each core
runs the full two-stream cross-attention block for its 2 images; no collectives.

Per-core algorithm (all layouts chosen so no transposes are ever needed):
  - depthwise 3x3 conv (+folded BN scale) via 9 fused scalar_tensor_tensor
    taps; channels on partitions, spatial flat on free dim. Full-res (q path)
    taps run on DVE in bf16 (2x mode); strided kv taps run on GpSimd (their
    stride-2 access can't use DVE 2x mode anyway).
  - pointwise convs on PE (bf16 operands, fp32 PSUM):
      Q:  [cin,pos] x WqT  -> qT [cout(head-major),pos]
      K:  [cin,pos] x WkvT -> kT [cout,pos]
      V:  x-stationary     -> v  [pos, cout]   (needed for attn@v contraction)
  - attention per head: dotsT = kT.T@qT on PE (kv on partitions, q on free),
    exp on ACT (logits are tiny -> no max subtraction) -> bf16 p, denominator
    via col-tiled ones-matmuls on PE + DVE reciprocal, oT = v.T@p on PE with
    head pairs packed in 128 partitions; normalization fused into the
    PSUM->SBUF eviction (DVE mult) -> bf16 oT.
  - V conv bias is folded into the output conv bias on the host (softmax rows
    sum to exactly 1), BN is folded into depthwise taps + pointwise bias.
  - output 1x1 conv on PE + bias on ACT, DMA out fp32.
"""

import numpy as np
import ml_dtypes
from contextlib import ExitStack

import concourse.bass as bass
import concourse.bacc as bacc
import concourse.tile as tile
import concourse.mybir as mybir
from concourse.bass_utils import run_bass_kernel_spmd

F32 = mybir.dt.float32
BF16 = mybir.dt.bfloat16
NPBF16 = ml_dtypes.bfloat16
FP8 = mybir.dt.float8e4
NPFP8 = ml_dtypes.float8_e4m3
DR = mybir.MatmulPerfMode.DoubleRow
AOP = mybir.AluOpType
AF = mybir.ActivationFunctionType

N_CORES = 8
B = 16
IMGS = B // N_CORES          # images per core
DIM = 256                    # conv in channels
HEADS, DH, INNER = 8, 64, 512
HW = 1024                    # 32*32 q positions
HWK = 256                    # 16*16 kv positions
EPS = 1e-5
SCALE = DH ** -0.5


def _r(ap):
    return ap


# ---------------------------------------------------------------------------
# device kernel
# ---------------------------------------------------------------------------

def _build_module():
    nc = bacc.Bacc("TRN2", target_bir_lowering=False, debug=False)

    def inp(name, shape, dt=F32):
        return nc.dram_tensor(name, shape, dt, kind="ExternalInput")

    x_d = [inp(f"xs{s}", [IMGS, DIM, 34 * 34], BF16) for s in range(2)]
    dgq_d = inp("dgq", [2, 128, 2, 9, 128], BF16)    # diag(dw), p-major for contiguous DMA
    dgkv_d = inp("dgkv", [2, 128, 2, 9, 128], BF16)
    wq_d = [inp(f"wq{s}", [DIM, INNER], BF16) for s in range(2)]      # [cin, cout]
    wkv_d = [inp(f"wkv{s}", [DIM, 2 * INNER], BF16) for s in range(2)]
    bq_d = [inp(f"bq{s}", [4, 128]) for s in range(2)]          # cout chunk-major
    bk_d = [inp(f"bk{s}", [4, 128]) for s in range(2)]
    wo_d = [inp(f"wo{s}", [INNER, DIM], BF16) for s in range(2)]      # [hd, c]
    bo_d = [inp(f"bo{s}", [2, 128]) for s in range(2)]
    ones_d = inp("ones_in", [128, 2, 192], FP8)
    out_d = nc.dram_tensor("out", [2, IMGS, DIM, HW], F32, kind="ExternalOutput")

    with tile.TileContext(nc) as tc, ExitStack() as ctx:
        const = ctx.enter_context(tc.tile_pool(name="const", bufs=1))
        xpool = ctx.enter_context(tc.tile_pool(name="xpool", bufs=2))
        yqpool = ctx.enter_context(tc.tile_pool(name="yqpool", bufs=2))
        ykpool = ctx.enter_context(tc.tile_pool(name="ykpool", bufs=2))
        qTpool = ctx.enter_context(tc.tile_pool(name="qTpool", bufs=8))
        kTpool = ctx.enter_context(tc.tile_pool(name="kTpool", bufs=8))
        vpool = ctx.enter_context(tc.tile_pool(name="vpool", bufs=4))
        pTpool = ctx.enter_context(tc.tile_pool(name="pTpool", bufs=16))
        Rpool = ctx.enter_context(tc.tile_pool(name="Rpool", bufs=2))
        oTpool = ctx.enter_context(tc.tile_pool(name="oTpool", bufs=4))
        outpool = ctx.enter_context(tc.tile_pool(name="outpool", bufs=2))
        psA = ctx.enter_context(tc.tile_pool(name="psA", bufs=2, space="PSUM"))
        psD = ctx.enter_context(tc.tile_pool(name="psD", bufs=2, space="PSUM"))
        psB = ctx.enter_context(tc.tile_pool(name="psB", bufs=2, space="PSUM"))

        # ---- constants -----------------------------------------------------
        wq_sb, wkv_sb, wo_sb, dwq_sb, dwkv_sb, bq_sb, bk_sb, bo_sb = (
            [], [], [], [], [], [], [], [])
        for s in range(2):
            wq_sb.append([const.tile([128, INNER], BF16, tag=f"wq{s}{k}", name=f"wq{s}{k}")
                          for k in range(2)])
            wkv_sb.append([const.tile([128, 2 * INNER], BF16, tag=f"wkv{s}{k}", name=f"wkv{s}{k}")
                           for k in range(2)])
            wo_sb.append([const.tile([128, DIM], BF16, tag=f"wo{s}{m}", name=f"wo{s}{m}")
                          for m in range(4)])
            for k in range(2):
                nc.sync.dma_start(out=wq_sb[s][k][:],
                                  in_=wq_d[s].ap()[k * 128:(k + 1) * 128, :])
                nc.sync.dma_start(out=wkv_sb[s][k][:],
                                  in_=wkv_d[s].ap()[k * 128:(k + 1) * 128, :])
            for m in range(4):
                nc.sync.dma_start(out=wo_sb[s][m][:],
                                  in_=wo_d[s].ap()[m * 128:(m + 1) * 128, :])
            dwq_sb.append(const.tile([128, 2, 9, 128], BF16, tag=f"dgq{s}", name=f"dgq{s}"))
            dwkv_sb.append(const.tile([128, 2, 9, 128], BF16, tag=f"dgkv{s}", name=f"dgkv{s}"))
            nc.sync.dma_start(out=dwq_sb[s][:], in_=dgq_d.ap()[s])
            nc.sync.dma_start(out=dwkv_sb[s][:], in_=dgkv_d.ap()[s])
            bq_sb.append(const.tile([128, 4], F32, tag=f"bq{s}", name=f"bq{s}"))
            bk_sb.append(const.tile([128, 4], F32, tag=f"bk{s}", name=f"bk{s}"))
            bo_sb.append(const.tile([128, 2], F32, tag=f"bo{s}", name=f"bo{s}"))
            nc.sync.dma_start(out=bq_sb[s][:], in_=bq_d[s].ap().rearrange("m p -> p m"))
            nc.sync.dma_start(out=bk_sb[s][:], in_=bk_d[s].ap().rearrange("m p -> p m"))
            nc.sync.dma_start(out=bo_sb[s][:], in_=bo_d[s].ap().rearrange("m p -> p m"))
        ones_sb = const.tile([128, 2, 192], FP8, tag="ones", name="ones")
        nc.sync.dma_start(out=ones_sb[:], in_=ones_d.ap())

        for img in range(IMGS):
            qT, kT, v = {}, {}, {}
            # ---- projections for both streams ------------------------------
            for s in range(2):
                # load x, depthwise
                yq, ykv = [], []
                for c in range(2):
                    x_t = xpool.tile([128, 34 * 34], BF16, tag="x", name="x")
                    nc.sync.dma_start(out=x_t[:],
                                      in_=x_d[s].ap()[img, c * 128:(c + 1) * 128, :])
                    x3 = x_t[:].rearrange("p (r q) -> p r q", r=34)
                    # depthwise 3x3 = 9 shifted diag-matmuls accumulating in PSUM
                    yq_t = yqpool.tile([128, HW], BF16, tag="yq", name="yq")
                    for half in range(2):
                        ps = psA.tile([128, 512], F32, tag="mm", name="mm")
                        for t in range(9):
                            ky, kx = t // 3, t % 3
                            nc.tensor.matmul(
                                out=ps[:],
                                lhsT=_r(dwq_sb[s][:, c, t, :]),
                                rhs=_r(x3[:, ky + 16 * half:ky + 16 * half + 16,
                                          kx:kx + 32]),
                                start=(t == 0), stop=(t == 8))
                        nc.vector.tensor_copy(yq_t[:, half * 512:(half + 1) * 512],
                                              ps[:])
                    yq.append(yq_t)

                    ykv_t = ykpool.tile([128, HWK], BF16, tag="ykv", name="ykv")
                    ps = psA.tile([128, 512], F32, tag="mm", name="mm")
                    for t in range(9):
                        ky, kx = t // 3, t % 3
                        nc.tensor.matmul(
                            out=ps[:, 0:256],
                            lhsT=_r(dwkv_sb[s][:, c, t, :]),
                            rhs=_r(x3[:, ky:ky + 32:2, kx:kx + 32:2]),
                            start=(t == 0), stop=(t == 8))
                    nc.vector.tensor_copy(ykv_t[:], ps[:, 0:256])
                    ykv.append(ykv_t)

                # Q pointwise -> qT8[g] [128,(hh,dl)  2,(j)  1024] fp8
                for g in range(2):
                    qT_t = qTpool.tile([128, 2, HW], FP8, tag="qT", name="qT")
                    for j in range(2):
                        m = 2 * g + j
                        for qh in range(2):
                            ps = psA.tile([128, 512], F32, tag="mm", name="mm")
                            for k in range(2):
                                nc.tensor.matmul(
                                    out=ps[:],
                                    lhsT=_r(wq_sb[s][k][:, m * 128:(m + 1) * 128]),
                                    rhs=_r(yq[k][:, qh * 512:(qh + 1) * 512]),
                                    start=(k == 0), stop=(k == 1))
                            if qh == 0:
                                nc.scalar.activation(
                                    out=qT_t[:, j, qh * 512:(qh + 1) * 512], in_=ps[:],
                                    func=AF.Identity, bias=bq_sb[s][:, m:m + 1], scale=1.0)
                            else:
                                nc.vector.tensor_scalar_add(
                                    qT_t[:, j, qh * 512:(qh + 1) * 512], ps[:],
                                    bq_sb[s][:, m:m + 1])
                    qT[(s, g)] = qT_t
                # K pointwise -> kT8[g] [128, 2, 256] fp8
                for g in range(2):
                    kT_t = kTpool.tile([128, 2, HWK], FP8, tag="kT", name="kT")
                    for j in range(2):
                        m = 2 * g + j
                        ps = psA.tile([128, HWK], F32, tag="mm", name="mm")
                        for k in range(2):
                            nc.tensor.matmul(
                                out=ps[:],
                                lhsT=_r(wkv_sb[s][k][:, m * 128:(m + 1) * 128]),
                                rhs=_r(ykv[k][:]),
                                start=(k == 0), stop=(k == 1))
                        nc.scalar.activation(out=kT_t[:, j, :], in_=ps[:], func=AF.Identity,
                                             bias=bk_sb[s][:, m:m + 1], scale=1.0)
                    kT[(s, g)] = kT_t
                # V pointwise (x-stationary): v[p] [128 pos, 512 cout]
                v_t = vpool.tile([128, 2, 512], FP8, tag="v", name="v")
                vo_t = vpool.tile([128, 2, 512], FP8, tag="vo", name="vo")
                for p in range(2):
                    ps = psA.tile([128, 512], F32, tag="mm", name="mm")
                    for k in range(2):
                        nc.tensor.matmul(
                            out=ps[:],
                            lhsT=_r(ykv[k][:, p * 128:(p + 1) * 128]),
                            rhs=_r(wkv_sb[s][k][:, INNER:2 * INNER]),
                            start=(k == 0), stop=(k == 1))
                    nc.vector.tensor_copy(v_t[:, p, :], ps[:])
                    vo4 = vo_t[:, p, :].rearrange("p (b w) -> p b w", b=4)
                    ps4 = ps.rearrange("p (b w) -> p b w", b=4)
                    nc.vector.tensor_scalar_mul(vo4[:, :, 0:64], ps4[:, :, 0:64], 0.0)
                    nc.vector.tensor_copy(vo4[:, :, 64:128], ps4[:, :, 64:128])
                v[s] = v_t
                v[(s, 'odd')] = vo_t

            # ---- attention blocks ------------------------------------------
            for a in range(2):          # output stream a: q from a, k/v from 1-a
                b = 1 - a
                pT = {}
                # dotsT + exp, per head pair
                for hp in range(4):
                    g = hp // 2
                    for kc in range(2):
                        dp = [psB.tile([128, HW], F32, tag="big", name="big") for _ in range(2)]
                        for j in range(2):        # j = head within pair
                            h = 2 * hp + j
                            hh = h % 4
                            for qh in range(2):
                                nc.tensor.matmul(
                                    out=dp[j][:, qh * 512:(qh + 1) * 512],
                                    lhsT=_r(kT[(b, g)][32 * hh:32 * (hh + 1), :,
                                                       kc * 128:(kc + 1) * 128]),
                                    rhs=_r(qT[(a, g)][32 * hh:32 * (hh + 1), :,
                                                      qh * 512:(qh + 1) * 512]),
                                    perf_mode=DR, start=True, stop=True)
                        for j in range(2):
                            h = 2 * hp + j
                            if (h, 0) not in pT:
                                pT8 = pTpool.tile([128, 2, HW], FP8, tag="pT", name="pT")
                                pT[('f', h)] = pT8
                                pT[(h, 0)], pT[(h, 1)] = pT8[:, 0, :], pT8[:, 1, :]
                            nc.scalar.activation(out=pT[(h, kc)], in_=dp[j][:],
                                                 func=AF.Exp, scale=SCALE)
                # denominators: zero-padded ones matmuls -> D replicated in
                # pair layout (rows 0-63 head 2hp, 64-127 head 2hp+1),
                # reciprocal straight off PSUM into the eviction multiplier
                dr = {}
                for hp in range(4):
                    dr_t = Rpool.tile([128, HW], F32, tag="R", name="R", bufs=3)
                    podd = pT[('f', 2 * hp + 1)]
                    pevn = pT[('f', 2 * hp)]
                    for qh in range(2):
                        d_t = psD.tile([128, 512], F32, tag="d", name="d")
                        nc.tensor.matmul(
                            out=d_t[:], lhsT=_r(ones_sb[:, :, 64:192]),
                            rhs=_r(podd[:, :, qh * 512:(qh + 1) * 512]),
                            perf_mode=DR, start=True, stop=False)
                        nc.tensor.matmul(
                            out=d_t[0:64, :], lhsT=_r(ones_sb[:, :, 0:64]),
                            rhs=_r(pevn[:, :, qh * 512:(qh + 1) * 512]),
                            perf_mode=DR, start=False, stop=True)
                        nc.vector.reciprocal_approx_fast(
                            out=dr_t[:, qh * 512:(qh + 1) * 512], in_=d_t[:])
                    dr[hp] = dr_t
                # oT per pair, odd head (zero-padded M=128, rows 64-127) first,
                # even head (M=64, rows 0-63) accumulates onto its zeros;
                # normalization fused into eviction
                oT = {}
                for hp in range(4):
                    po = psB.tile([128, HW], F32, tag="big", name="big")
                    podd = pT[('f', 2 * hp + 1)]
                    pevn = pT[('f', 2 * hp)]
                    for qh in range(2):
                        nc.tensor.matmul(
                            out=po[:, qh * 512:(qh + 1) * 512],
                            lhsT=_r(v[(b, 'odd')][:, :, 128 * hp:128 * (hp + 1)]),
                            rhs=_r(podd[:, :, qh * 512:(qh + 1) * 512]),
                            perf_mode=DR, start=True, stop=False)
                        nc.tensor.matmul(
                            out=po[0:64, qh * 512:(qh + 1) * 512],
                            lhsT=_r(v[b][:, :, 128 * hp:128 * hp + 64]),
                            rhs=_r(pevn[:, :, qh * 512:(qh + 1) * 512]),
                            perf_mode=DR, start=False, stop=True)
                    oT_t = oTpool.tile([128, HW], BF16, tag="oT", name="oT")
                    nc.vector.tensor_mul(oT_t[:], po[:], dr[hp][:])
                    oT[hp] = oT_t
                # output 1x1 conv + bias
                for cc in range(2):
                    out_t = outpool.tile([128, HW], F32, tag="out", name="out")
                    for qh in range(2):
                        ps = psA.tile([128, 512], F32, tag="mm", name="mm")
                        for hp in range(4):
                            nc.tensor.matmul(
                                out=ps[:],
                                lhsT=_r(wo_sb[a][hp][:, cc * 128:(cc + 1) * 128]),
                                rhs=_r(oT[hp][:, qh * 512:(qh + 1) * 512]),
                                start=(hp == 0), stop=(hp == 3))
                        nc.scalar.activation(
                            out=out_t[:, qh * 512:(qh + 1) * 512], in_=ps[:],
                            func=AF.Identity, bias=bo_sb[a][:, cc:cc + 1], scale=1.0)
                    nc.sync.dma_start(
                        out=out_d.ap()[a, img, cc * 128:(cc + 1) * 128, :],
                        in_=out_t[:])
    nc.compile()
    return nc


_MODULE = None


def _get_module():
    global _MODULE
    if _MODULE is None:
        _MODULE = _build_module()
    return _MODULE


# ---------------------------------------------------------------------------
# host side: BN folding + sharding + launch
# ---------------------------------------------------------------------------

def _fold(inputs, p):
    dw = np.asarray(inputs[p + '_dw'], np.float32)[:, 0]        # [256,3,3]
    g = np.asarray(inputs[p + '_g'], np.float32)
    b_ = np.asarray(inputs[p + '_b'], np.float32)
    rm = np.asarray(inputs[p + '_rm'], np.float32)
    rv = np.asarray(inputs[p + '_rv'], np.float32)
    pw = np.asarray(inputs[p + '_pw'], np.float32)[:, :, 0, 0]  # [cout, 256]
    inv = g / np.sqrt(rv + EPS)
    dw_eff = (dw * inv[:, None, None]).reshape(DIM, 9)
    bias = pw @ (b_ - rm * inv)
    return dw_eff, pw.T.copy(), bias                             # WT [256, cout]


def host_arrays(inputs):
    """Folded per-core-constant DRAM tensors (same on every core)."""
    ones1 = np.concatenate([np.ones((128, 1, 64), np.float32),
                            np.zeros((128, 1, 64), np.float32),
                            np.ones((128, 1, 64), np.float32)], axis=2)
    h = {'ones_in': np.repeat(ones1, 2, axis=1)}
    bf16_keys = {'ones_in', 'dgq', 'dgkv'}
    bv = {}
    dgq, dgkv = {}, {}
    for s, qp, kvp in ((0, 'q1', 'kv1'), (1, 'q2', 'kv2')):
        dwq, WqT, bq = _fold(inputs, qp)
        dwkv, WkvT, bkv = _fold(inputs, kvp)
        dgq[s] = dwq.reshape(2, 128, 9)
        dgkv[s] = dwkv.reshape(2, 128, 9)
        # column order for DoubleRow dots: m=(g,j) chunk, row = 32*(h%4)+dl,
        # holding cout (4g+h%4)*64 + 32j + dl
        perm = np.array([(4 * g + hh) * 64 + 32 * j + dl
                         for g in range(2) for j in range(2)
                         for hh in range(4) for dl in range(32)])
        h[f'wq{s}'] = np.ascontiguousarray(WqT[:, perm])
        WkvT2 = WkvT.copy()
        WkvT2[:, :INNER] = WkvT[:, perm]
        h[f'wkv{s}'] = np.ascontiguousarray(WkvT2)
        bf16_keys |= {f'wq{s}', f'wkv{s}'}
        h[f'bq{s}'] = bq[perm].reshape(4, 128)
        h[f'bk{s}'] = bkv[perm].reshape(4, 128)
        bv[s] = bkv[INNER:]
    for nm, dg in (('dgq', dgq), ('dgkv', dgkv)):
        arr = np.zeros((2, 2, 9, 128, 128), np.float32)
        for s in range(2):
            for c in range(2):
                for t in range(9):
                    np.fill_diagonal(arr[s, c, t], dg[s][c, :, t])
        h[nm] = np.ascontiguousarray(arr.transpose(0, 3, 1, 2, 4))
    for s, op in ((0, 'out1'), (1, 'out2')):
        Wout = np.asarray(inputs[op + '_w'], np.float32)[:, :, 0, 0]  # [256, 512]
        bo = np.asarray(inputs[op + '_b'], np.float32) + Wout @ bv[1 - s]
        h[f'wo{s}'] = np.ascontiguousarray(Wout.T)
        bf16_keys.add(f'wo{s}')
        h[f'bo{s}'] = bo.reshape(2, 128)
    out = {}
    for k, a in h.items():
        dt = NPFP8 if k == 'ones_in' else (NPBF16 if k in bf16_keys else np.float32)
        out[k] = np.ascontiguousarray(a, dtype=dt)
    return out


def make_in_maps(inputs):
    h = host_arrays(inputs)
    def pad_x(a):
        a = np.asarray(a, np.float32).reshape(B, DIM, 32, 32)
        p = np.zeros((B, DIM, 34, 34), np.float32)
        p[:, :, 1:33, 1:33] = a
        return p.reshape(B, DIM, 34 * 34).astype(NPBF16)
    x1 = pad_x(inputs['x1'])
    x2 = pad_x(inputs['x2'])
    maps = []
    for c in range(N_CORES):
        m = dict(h)
        m['xs0'] = np.ascontiguousarray(x1[c * IMGS:(c + 1) * IMGS])
        m['xs1'] = np.ascontiguousarray(x2[c * IMGS:(c + 1) * IMGS])
        maps.append(m)
    return maps


def gather_out(core_outs):
    """core_outs: list of [2, IMGS, 256, 1024] -> [2, B, 256, 32, 32]."""
    full = np.concatenate([np.asarray(o) for o in core_outs], axis=1)
    return np.ascontiguousarray(full.reshape(2, B, DIM, 32, 32).astype(np.float32))


def kernel(**inputs):
    nc = _get_module()
    in_maps = make_in_maps(inputs)
    res = run_bass_kernel_spmd(nc, in_maps, list(range(N_CORES)))
    return gather_out([r['out'] for r in res.results])


if __name__ == '__main__':
    nc = _build_module()
    print("module built OK")
